# revision 25
# baseline (speedup 1.0000x reference)
"""Cross-entropy loss (nn_CrossEntropyLoss) on 8 Trainium2 NeuronCores.

Reference computation (full shapes):
    predicts: [4096, 32000] f32, targets: [4096] int64
    loss = mean_i( log(sum_j exp(predicts[i, j])) - predicts[i, targets[i]] )

Strategy (v7): data-parallel over batch; fp8 stream; exp work split across
THREE engines (ACT exact exp, DVE bit-trick exp, PE for the row sums).
  - The device computes only the per-row sum-of-exps; the O(B) rest
    (picked-logit gather, log, mean) runs on the host, like the
    baseline's host-side mean.
  - predicts is cast to fp8 (e4m3) on the host before upload, cutting
    HBM traffic per core to 16.4 MB (loss rel err ~1e-6 from
    quantization; tolerance 2e-2).
  - Per core [512, 32000] shard, two complementary column regions:
    * ACT region (12672 cols, row-major [512, 12672]): streamed as
      [128 x chunk] tiles per 128-row block; ACT computes exact exp
      (fp8 in -> f16 out; NOT in-place: exp(6.2) ~ 493 overflows e4m3)
      with accum_out row-sums. ~44us.
    * PE region (19328 cols, uploaded TRANSPOSED as [128, 151*512] so
      partition p holds column 12672+g*128+p for all 151 groups g):
      DVE runs the Schraudolph bit-trick exp -- y = round(x*2^10/ln2 +
      B16) as int16; bitcast to f16 it is exp(x) with ~3% mean-zero
      noise that averages out over the row sum (numpy-validated final
      rel err ~5e-4). One 2x-mode tensor_scalar pass, ~42us. The row
      sums fall out of the TENSOR engine: in this layout a row sum is a
      sum over partitions, so ones[128,1]^T @ ev[:, g*512:(g+1)*512]
      matmuls accumulate all 151 groups into one PSUM [1, 512] tile,
      ~33us on an otherwise idle engine.
  - Outputs: ACT chunk sums [128, 10] f32 + PE row sums [1, 512] f32.
Host: S_row = act part + pe part, loss = mean(log(S) - picked).
"""

import sys

import numpy as np

sys.path.insert(0, "/opt/trn_rl_repo")

BATCH = 4096
C = 32000
NCORES = 8
R = BATCH // NCORES  # 512 rows per core
P = 128
NBLK = R // P  # 4 row blocks per core

C_ACT = 11904  # row-major columns, exact exp on ACT
C_PE = C - C_ACT  # 20096 transposed columns, DVE bit-trick + PE sums
N_GROUPS = C_PE // P  # 157 column groups of 128
G_PER_TS = 8  # groups per DVE tensor_scalar ([128, 4096])
G_PER_DMA = 16  # groups per transposed-tile DMA ([128, 8192] fp8, 1 MB)

# ACT chunk widths per block: ramp up in block 0, taper down in block 3
ACT_WIDTHS = [
    [2000, 5904, 4000],
    [8000, 3904],
    [8000, 3904],
    [7904, 3000, 1000],
]
N_ACT_CHUNKS = sum(len(w) for w in ACT_WIDTHS)  # 10

_CACHE: dict = {}

# Schraudolph exp constants in fp16-bit space:
#   bits_f16(exp(x)) ~= round(A16*x + B16)
A16 = 1024.0 / float(np.log(2.0))
B16 = 15.0 * 1024 - 0.043677448 * 1024  # mean-centering constant


def _patch_act_tables():
    """Make the act-table pass pick `natural_log_exp_and_others` for Exp so
    the kernel needs exactly one ACT_TABLE_LOAD."""
    import concourse.bacc as bacc
    import concourse.hw_specs as hw_specs
    from concourse import mybir

    orig = hw_specs.get_activation_tables("gen3")
    patched = {}
    for name, funcs in orig.items():
        f = set(funcs)
        if name != "natural_log_exp_and_others":
            f.discard(mybir.ActivationFunctionType.Exp)
            f.discard(mybir.ActivationFunctionType.Ln)
        patched[name] = f
    saved = bacc.get_activation_tables
    bacc.get_activation_tables = lambda arch: patched
    return saved


def _build_nc():
    import concourse.bacc as bacc
    import concourse.tile as tile
    from concourse import bass, mybir

    restore_tables = _patch_act_tables()
    nc = bacc.Bacc(
        "TRN2", target_bir_lowering=False, debug=False, num_devices=NCORES
    )
    xr = nc.dram_tensor("xr", [R, C_ACT], mybir.dt.float8e4, kind="ExternalInput")
    xt = nc.dram_tensor(
        "xt", [P, N_GROUPS * R], mybir.dt.float8e4, kind="ExternalInput"
    )
    sums_a = nc.dram_tensor(
        "sums_a", [P, N_ACT_CHUNKS], mybir.dt.float32, kind="ExternalOutput"
    )
    spe = nc.dram_tensor("spe", [1, R], mybir.dt.float32, kind="ExternalOutput")

    # interleave plan: transposed tiles are spread between ACT chunks so
    # DVE and PE run concurrently with ACT for the whole stream, and the
    # stream ends on the small final ACT chunk (short tail). Entry k of
    # tiles_after_chunk = how many transposed-tile DMAs (16 groups each)
    # to issue after ACT chunk k (10 chunks, 10 tile DMAs, front-loaded).
    tiles_after_chunk = [2, 1, 1, 1, 1, 1, 1, 1, 1, 0]
    n_tiles = (N_GROUPS + G_PER_DMA - 1) // G_PER_DMA

    with tile.TileContext(nc) as tc:
        with (
            tc.tile_pool(name="xr8", bufs=4) as xrpool,
            tc.tile_pool(name="xt8", bufs=4) as xtpool,
            tc.tile_pool(name="eact", bufs=2) as apool,
            tc.tile_pool(name="edve", bufs=3) as epool,
            tc.tile_pool(name="small", bufs=1) as spool,
            tc.psum_pool(name="ps", bufs=1) as ppool,
        ):
            sums_t = spool.tile([P, N_ACT_CHUNKS], mybir.dt.float32, tag="sums")
            spe_t = spool.tile([1, R], mybir.dt.float32, tag="spe")
            ones_t = spool.tile([P, 1], mybir.dt.float16, tag="ones")
            ps = ppool.tile([1, R], mybir.dt.float32, tag="ps")
            nc.vector.memset(ones_t[:], 1.0)

            gi = 0  # global PE group index
            ti = 0  # tile index
            ci = 0  # global ACT chunk index

            def issue_pe_tile():
                nonlocal gi, ti
                g0 = ti * G_PER_DMA
                ng = min(G_PER_DMA, N_GROUPS - g0)
                w = ng * R
                xtile = xtpool.tile([P, G_PER_DMA * R], mybir.dt.float8e4, tag="xt")
                # transposed tiles ride the SWDGE (gpsimd) queue, in
                # parallel with the ACT chunks on the sync HWDGE queue:
                # two queues keep more reads in flight, hiding per-DMA
                # completion latency and improving this core's HBM share
                nc.gpsimd.dma_start(out=xtile[:, :w], in_=xt[:, g0 * R : g0 * R + w])
                # split the 16-group DMA tile into 8-group tensor_scalars
                # so PE can start on the first half while DVE does the rest
                for s0 in range(0, ng, G_PER_TS):
                    ns = min(G_PER_TS, ng - s0)
                    sw = ns * R
                    et = epool.tile([P, G_PER_TS * R], mybir.dt.int16, tag="et")
                    nc.vector.tensor_scalar(
                        out=et[:, :sw],
                        in0=xtile[:, s0 * R : s0 * R + sw],
                        scalar1=A16,
                        scalar2=B16,
                        op0=mybir.AluOpType.mult,
                        op1=mybir.AluOpType.add,
                    )
                    ev = et[:, :sw].bitcast(mybir.dt.float16)
                    for g in range(ns):
                        nc.tensor.matmul(
                            out=ps[:],
                            lhsT=ones_t[:],
                            rhs=ev[:, g * R : (g + 1) * R],
                            start=(gi == 0),
                            stop=(gi == N_GROUPS - 1),
                        )
                        gi += 1
                ti += 1

            for b in range(NBLK):
                col = 0
                for w in ACT_WIDTHS[b]:
                    xtile = xrpool.tile([P, 8000], mybir.dt.float8e4, tag="xr")
                    nc.sync.dma_start(
                        out=xtile[:, :w],
                        in_=xr[b * P : (b + 1) * P, col : col + w],
                    )
                    at = apool.tile([P, 8000], mybir.dt.float16, tag="at")
                    nc.scalar.activation(
                        out=at[:, :w],
                        in_=xtile[:, :w],
                        func=mybir.ActivationFunctionType.Exp,
                        accum_out=sums_t[:, ci : ci + 1],
                    )
                    col += w
                    for _ in range(tiles_after_chunk[ci]):
                        if ti < n_tiles:
                            issue_pe_tile()
                    ci += 1
            while ti < n_tiles:
                issue_pe_tile()

            nc.vector.tensor_copy(out=spe_t[:], in_=ps[:])
            nc.sync.dma_start(out=spe[:, :], in_=spe_t[:])
            nc.sync.dma_start(out=sums_a[:, :], in_=sums_t[:])
    nc.compile()
    import concourse.bacc as bacc_mod

    bacc_mod.get_activation_tables = restore_tables
    return nc


def get_nc():
    if "nc" not in _CACHE:
        _CACHE["nc"] = _build_nc()
    return _CACHE["nc"]


def make_in_maps(predicts: np.ndarray, targets: np.ndarray) -> list[dict]:
    """Shard per core; cast to fp8 e4m3; build the transposed PE region."""
    import ml_dtypes

    predicts = np.ascontiguousarray(predicts, dtype=np.float32)
    x8 = predicts.astype(ml_dtypes.float8_e4m3)
    in_maps = []
    for c in range(NCORES):
        shard = x8[c * R : (c + 1) * R]
        xr = np.ascontiguousarray(shard[:, :C_ACT])
        # xt[p, g*R + r] = shard[r, C_ACT + g*128 + p]
        xt = np.ascontiguousarray(
            shard[:, C_ACT:].reshape(R, N_GROUPS, P).transpose(2, 1, 0).reshape(P, -1)
        )
        in_maps.append({"xr": xr, "xt": xt})
    return in_maps


def kernel(predicts: np.ndarray, targets: np.ndarray) -> np.ndarray:
    from concourse.bass_utils import run_bass_kernel_spmd

    nc = get_nc()
    predicts = np.ascontiguousarray(predicts, dtype=np.float32)
    targets = np.asarray(targets).astype(np.int64)
    in_maps = make_in_maps(predicts, targets)
    res = run_bass_kernel_spmd(nc, in_maps, list(range(NCORES)))

    # chunk -> block mapping for the ACT sums
    blk_of_chunk = []
    for b in range(NBLK):
        blk_of_chunk += [b] * len(ACT_WIDTHS[b])

    total = np.float64(0.0)
    for c in range(NCORES):
        sa = np.asarray(res.results[c]["sums_a"], dtype=np.float64)  # [128, 10]
        sp = np.asarray(res.results[c]["spe"], dtype=np.float64)[0]  # [512]
        s_act = np.zeros((NBLK, P))
        for j, b in enumerate(blk_of_chunk):
            s_act[b] += sa[:, j]
        s_row = s_act.reshape(R) + sp  # row r = b*128 + p
        total += np.log(s_row).sum()
    picked = predicts[np.arange(BATCH), targets].astype(np.float64).sum()
    return np.asarray((total - picked) / BATCH, dtype=np.float32)
